# revision 2
# baseline (speedup 1.0000x reference)
"""Trainium2 Bass kernel for nn_Loss_refine_8778913153424 (ADD/ADD-S pose loss).

Data-parallel over batch: 32 batches -> 8 NeuronCores x 4 batches.

Math: for each batch, with R = quat_to_mat(pred_r), t = pred_t,
  pred_m = R p_m + t  (p = model_points).  Distances are rigid-invariant:
  ||pred_m - target_n|| = ||p_m - w_n||  with  w_n = R^T (target_n - t).
So the device computes cdist(p, w) via a K=15 "lifted" matmul:
  d2[m,n] = p2[m]*1 + 1*w2[n] + (-2 p_m) . w_n
with each operand split hi/lo in bf16 (split-precision: A.B ~= Ah.Bh +
Al.Bh + Ah.Bl, error ~2^-16 relative) so the PE runs at 1 col/cycle.

Per 128-row tile the 2048-wide d2 row block lands in 4 PSUM banks; drain is
split between ScalarE (copies banks 2-3 to SBUF) and VectorE (a custom DVE op
min(Src0,Src1) with accum=MIN reads banks 0-1 from PSUM while folding the
ScalarE copy as its second stream, emitting the per-row min directly).
sqrt + per-batch means run on ACT (+ a ones-matmul partition reduction).
Host does only: input prep/sharding, and the final 32-way select/mean.
"""

import os
import numpy as np
import ml_dtypes

import concourse.bass as bass  # noqa: F401  (engine classes)
import concourse.bacc as bacc
import concourse.tile as tile
import concourse.mybir as mybir
import concourse.dve_ops as dve_ops_mod
from concourse.dve_spec import Spec, Src0, Src1, C0, minn, lower, AluOp
from concourse.dve_uop import DveOpSpec
from concourse.bass_utils import run_bass_kernel_spmd

B, M, NCORES = 32, 2048, 8
NB = B // NCORES          # 4 batches per core
K = 15                    # lifted + split-precision contraction dim
RT = M // 128             # 16 row tiles
BF16 = ml_dtypes.bfloat16
F32 = mybir.dt.float32
SQ_SCALE = float(2.0 ** -22)   # (1/2048)^2, exact; sqrt(s^2 x) = s sqrt(x)
SEED = 3.0e38

_MIN2_NAME = "TT_MIN_ACC_K8778"
LAST_RESULTS = None


def _register_min2():
    for op in dve_ops_mod.OPS:
        if op.name == _MIN2_NAME:
            return op

    def _ref(in0, in1, s0, s1, imm2):
        b = np.minimum(np.asarray(in0, np.float32), np.asarray(in1, np.float32))
        b = b.astype(np.float32)
        acc = np.minimum(
            np.float32(s0), b.reshape(b.shape[0], -1).min(axis=-1, keepdims=True)
        ).astype(np.float32)
        return b, acc

    spec = Spec(body=minn(Src0, Src1), accum=AluOp.MIN, accum_init=C0, reference=_ref)
    row = dve_ops_mod._CUSTOM_DVE_ROW_BASE + len(dve_ops_mod.OPS)
    shas = {}
    for ver in ("v3", "v4"):
        uops = lower(spec, ver=ver)
        shas[ver] = DveOpSpec(
            name=_MIN2_NAME, opcode=row, uops=uops, rd1_en=True
        ).sha(ver)
    op = dve_ops_mod.DveOp(_MIN2_NAME, spec, subdim=False, uops_sha=shas)
    dve_ops_mod.OPS.append(op)
    dve_ops_mod.CUSTOM_DVE_SPECS[op.name] = op.spec
    dve_ops_mod._SUB_OPCODE_FOR_NAME[op.name] = row
    return op


MIN2 = _register_min2()

_CACHE = {}


def build_bass():
    if "nc" in _CACHE:
        return _CACHE["nc"]
    nc = bacc.Bacc(
        trn_type="TRN2",
        target_bir_lowering=False,
        debug=False,
        enable_asserts=False,
    )
    a16_d = nc.dram_tensor("a16", [NB, K, M], mybir.dt.bfloat16, kind="ExternalInput").ap()
    b16_d = nc.dram_tensor("b16", [NB, K, M], mybir.dt.bfloat16, kind="ExternalInput").ap()
    pw_d = nc.dram_tensor("pw", [NB, 2, 128, 48], F32, kind="ExternalInput").ap()
    ones_d = nc.dram_tensor("ones", [128, 1], F32, kind="ExternalInput").ap()
    res_d = nc.dram_tensor("res", [2 * NB, 1], F32, kind="ExternalOutput").ap()

    with tile.TileContext(nc) as tc:
        with (
            tc.tile_pool(name="ops", bufs=2) as opool,
            tc.tile_pool(name="pwp", bufs=2) as pwpool,
            tc.tile_pool(name="cpp", bufs=3) as cpool,
            tc.tile_pool(name="scr", bufs=3) as spool,
            tc.tile_pool(name="stats", bufs=2 * NB) as stpool,
            tc.tile_pool(name="fin", bufs=1) as finpool,
        ):
            ones_t = finpool.tile([128, 1], F32, tag="ones")
            nc.sync.dma_start(ones_t[:], ones_d[:, :])
            sums_all = finpool.tile([128, 2 * NB], F32, tag="sums")
            mins_list, d2a_list = [], []
            with tc.tile_pool(name="psA", bufs=2, space="PSUM") as psA:
                for b in range(NB):
                    Alhs = opool.tile([K, M], mybir.dt.bfloat16, tag="Alhs")
                    Brhs = opool.tile([K, M], mybir.dt.bfloat16, tag="Brhs")
                    nc.sync.dma_start(Alhs[:], a16_d[b])
                    nc.sync.dma_start(Brhs[:], b16_d[b])
                    p_t = pwpool.tile([128, 48], F32, tag="p")
                    w_t = pwpool.tile([128, 48], F32, tag="w")
                    nc.sync.dma_start(p_t[:], pw_d[b, 0])
                    nc.sync.dma_start(w_t[:], pw_d[b, 1])

                    # ADD branch: per-point ||p_m - w_m||^2 -> [128,16]
                    diff = spool.tile([128, 48], F32, tag="diff")
                    nc.vector.tensor_tensor(diff[:], p_t[:], w_t[:], op=mybir.AluOpType.subtract)
                    sq = spool.tile([128, 48], F32, tag="sq")
                    nc.vector.tensor_tensor(sq[:], diff[:], diff[:], op=mybir.AluOpType.mult)
                    d2a = stpool.tile([128, 16], F32, tag="stats")
                    nc.vector.tensor_reduce(
                        d2a[:],
                        sq[:].rearrange("p (g t) -> p g t", t=3),
                        axis=mybir.AxisListType.X,
                        op=mybir.AluOpType.add,
                    )
                    d2a_list.append(d2a)

                    mins_b = stpool.tile([128, 16], F32, tag="stats")
                    mins_list.append(mins_b)
                    for r in range(RT):
                        ps = psA.tile([128, M], F32, tag="d2")
                        for c in range(4):
                            nc.tensor.matmul(
                                ps[:, 512 * c:512 * (c + 1)],
                                Alhs[:, 128 * r:128 * (r + 1)],
                                Brhs[:, 512 * c:512 * (c + 1)],
                                start=True,
                                stop=True,
                            )
                        cp = cpool.tile([128, 1024], F32, tag="cp")
                        nc.scalar.copy(cp[:], ps[:, 1024:2048])
                        scr = spool.tile([128, 1024], F32, tag="mscr")
                        nc.vector._custom_dve(
                            MIN2,
                            out=scr[:],
                            in0=ps[:, 0:1024],
                            in1=cp[:],
                            s0=SEED,
                            accum_out=mins_b[:, r:r + 1],
                        )

            # Final: clamp, sqrt (scaled so accum == mean), per-batch sums.
            for b in range(NB):
                mcl = spool.tile([128, 16], F32, tag="mcl")
                nc.vector.tensor_scalar_max(mcl[:], mins_list[b][:], 0.0)
                s16 = spool.tile([128, 16], F32, tag="s16")
                nc.scalar.activation(
                    s16[:], mcl[:], mybir.ActivationFunctionType.Sqrt,
                    scale=SQ_SCALE, accum_out=sums_all[:, 2 * b:2 * b + 1],
                )
                s16b = spool.tile([128, 16], F32, tag="s16b")
                nc.scalar.activation(
                    s16b[:], d2a_list[b][:], mybir.ActivationFunctionType.Sqrt,
                    scale=SQ_SCALE, accum_out=sums_all[:, 2 * b + 1:2 * b + 2],
                )
            with tc.tile_pool(name="psB", bufs=1, space="PSUM") as psB:
                out_ps = psB.tile([2 * NB, 1], F32, tag="out")
                nc.tensor.matmul(out_ps[:], sums_all[:], ones_t[:], start=True, stop=True)
                res_s = finpool.tile([2 * NB, 1], F32, tag="res")
                nc.vector.tensor_copy(res_s[:], out_ps[:])
                nc.sync.dma_start(res_d[:, :], res_s[:])

    nc.compile()
    _CACHE["nc"] = nc
    return nc


def _host_prep(pred_r, pred_t, model_points, target):
    q = pred_r / (np.linalg.norm(pred_r, axis=1, keepdims=True) + 1e-8)
    w_, x_, y_, z_ = q[:, 0], q[:, 1], q[:, 2], q[:, 3]
    R = np.stack(
        [
            np.stack([1 - 2 * (y_ * y_ + z_ * z_), 2 * (x_ * y_ - z_ * w_), 2 * (x_ * z_ + y_ * w_)], axis=-1),
            np.stack([2 * (x_ * y_ + z_ * w_), 1 - 2 * (x_ * x_ + z_ * z_), 2 * (y_ * z_ - x_ * w_)], axis=-1),
            np.stack([2 * (x_ * z_ - y_ * w_), 2 * (y_ * z_ + x_ * w_), 1 - 2 * (x_ * x_ + y_ * y_)], axis=-1),
        ],
        axis=-2,
    ).astype(np.float32)  # (B,3,3)
    v = (target - pred_t[:, None, :]).astype(np.float32)
    w = np.einsum("bnj,bji->bni", v, R).astype(np.float32)  # w_n = R^T v_n
    p = model_points.astype(np.float32)
    p2 = (p * p).sum(-1)
    w2 = (w * w).sum(-1)
    onesM = np.ones_like(p2)
    A = np.stack([p2, onesM, -2 * p[..., 0], -2 * p[..., 1], -2 * p[..., 2]], axis=1)
    Bm = np.stack([onesM, w2, w[..., 0], w[..., 1], w[..., 2]], axis=1)
    Ah = A.astype(BF16)
    Al = (A - Ah.astype(np.float32)).astype(BF16)
    Bh = Bm.astype(BF16)
    Bl = (Bm - Bh.astype(np.float32)).astype(BF16)
    A16 = np.concatenate([Ah, Al, Ah], axis=1)  # (B,15,M)
    B16 = np.concatenate([Bh, Bh, Bl], axis=1)
    pw = np.stack(
        [p.reshape(B, 128, 48), w.reshape(B, 128, 48)], axis=1
    ).astype(np.float32)  # (B,2,128,48)
    return A16, B16, pw


def kernel(**inputs):
    global LAST_RESULTS
    pred_r = np.asarray(inputs["pred_r"], np.float32)
    pred_t = np.asarray(inputs["pred_t"], np.float32)
    model_points = np.asarray(inputs["model_points"], np.float32)
    target = np.asarray(inputs["target"], np.float32)
    idx = np.asarray(inputs["idx"]).astype(np.int64)

    A16, B16, pw = _host_prep(pred_r, pred_t, model_points, target)
    nc = build_bass()
    ones = np.ones((128, 1), np.float32)
    in_maps = []
    for c in range(NCORES):
        sl = slice(NB * c, NB * (c + 1))
        in_maps.append(
            {"a16": np.ascontiguousarray(A16[sl]),
             "b16": np.ascontiguousarray(B16[sl]),
             "pw": np.ascontiguousarray(pw[sl]),
             "ones": ones}
        )
    trace = bool(os.environ.get("K_TRACE"))
    results = run_bass_kernel_spmd(
        nc, in_maps, core_ids=list(range(NCORES)), trace=trace
    )
    LAST_RESULTS = results
    adds = np.empty(B, np.float32)
    add = np.empty(B, np.float32)
    for c in range(NCORES):
        r = np.asarray(results.results[c]["res"], np.float32).reshape(2 * NB)
        for b in range(NB):
            adds[NB * c + b] = r[2 * b]
            add[NB * c + b] = r[2 * b + 1]
    sym = idx <= 3  # SYM_IDS = {0,1,2,3}; idx in [0,8)
    dists = np.where(sym, adds, add).astype(np.float32)
    loss = np.float32(dists.mean())
    return loss, dists
